# revision 12
# baseline (speedup 1.0000x reference)
"""Trainium2 kernel for nn_InterpolatorMaskArgs (embedding_lookup, memory regime).

reference computes:  ind = floor((x[0]-X0)/DX);  res = sum(roll(mask, ind) * yOrig)
i.e. a full O(N) dot product between yOrig and the rolled mask, with an
out-of-range guard on x.  Measured ~39-40 us HW exec (baseline was ~66 us).

Strategy (matches the sharding hint):
- 1-D shard yOrig/mask along N across the 8 cores; the roll is resolved at
  shard time (mod-N wraparound == the halo exchange, done while packing),
  and the final reduce is a host-side sum of the 8*128*NTILES fp32 partials.
- Both inputs are cast to fp16 on the host (tolerance is 2e-2; measured
  end-to-end rel err ~1.1e-3).  8 MiB/core of DMA; the binding resource is
  the SDMA aggregate at ~340-400 GB/s (fp8 variants don't help: cast-DMA
  still writes fp16 to SBUF, and fp8-consuming DVE ops measured 4-5x slower
  than fp16 2x mode).
- y and mask are packed per tile so ONE DMA per tile brings both halves
  (rows of 4*T_k bytes -> 8 KiB descriptors for 2048-wide tiles; <8 KiB
  descriptors measurably collapse ring throughput).
- Tiles alternate between the sync and scalar HWDGE rings so consecutive
  transfers overlap their ~1.6 us completion-receipt overheads.  No SWDGE
  (gpsimd) ring: it starts ~3.5 us late and its Q7 descriptor traffic slows
  the compute engines.  One semaphore PER TRANSFER (a cumulative sem races:
  SDMA engines interleave across in-flight transfers).
- Full shard lives in SBUF (8 of 24 MiB); no buffer recycling gates the
  rings; VectorE chases the per-transfer semaphores.
- Per tile: VectorE fp16 in-place product (DVE 2x mode).  The free-dim
  reduction to fp32 partials is split so no engine exceeds the stream time:
  bulk tiles on the otherwise idle ScalarE (activation-Copy accum_out),
  early-mid tiles plus the tapered 512-wide tail tiles on VectorE
  (tensor_scalar accum; the tail ones are guarded by a dummy op ordered
  after the compiler-emitted accumulator flush before the out-DMA reads).
  (The fused tensor_tensor_reduce would do mul+reduce in one op but this
  walrus build rejects its ISA encoding.)
"""
import numpy as np

import concourse.bass as bass
import concourse.mybir as mybir
from concourse.bass_utils import run_bass_kernel_spmd

N = 16777216
X0 = 0.0
DX = 1.0
XMAX = X0 + (N - 1) * DX

NCORES = 8
P = 128
S = N // NCORES
F = S // P

T_K = [2048] * 7 + [1024, 512, 512]      # tile widths, sum = F
NTILES = len(T_K)
STARTS = np.cumsum([0] + T_K).tolist()    # in the logical F dimension
OFFS = [2 * s for s in STARTS]            # in the packed 2F-wide SBUF/DRAM
W2 = 2 * F                                # packed row width
RING_S = [0, 2, 4, 6, 8]                  # tiles on the sync ring
RING_A = [1, 3, 5, 7, 9]                  # tiles on the scalar ring
# Tiles reduced on VectorE (tensor_scalar accum).  Mid-stream ONLY: the DVE
# holds the accumulation in a cache that an auto-emitted DVE_READ_ACCUMULATOR
# flushes to SBUF after the sem-incrementing instruction — a vector-reduced
# LAST tile races the out-DMA against that flush (observed garbage columns).
VEC_ACC = (2, 4, 8, 9)
TAIL_VEC = (8, 9)   # tail TS tiles: inc deferred to the flush-guard dummy

_CACHED_NC = None


def _build_nc():
    nc = bass.Bass(trn_type="TRN2")
    f16 = mybir.dt.float16
    f32 = mybir.dt.float32
    ym = nc.dram_tensor("ym", [P, W2], f16, kind="ExternalInput")
    out = nc.dram_tensor("out", [P, NTILES], f32, kind="ExternalOutput")

    import contextlib
    with contextlib.ExitStack() as stack:
        block = stack.enter_context(nc.Block())
        # One semaphore PER TRANSFER: with several transfers pipelined on a
        # ring, a cumulative sem at 16*(pos+1) does NOT imply transfer pos
        # finished (one SDMA engine can run ahead on transfer pos+1 while
        # another lags on pos) — observed as corrupted tiles under tracing.
        tsem = [stack.enter_context(nc.semaphore(f"dt{i}")) for i in range(NTILES)]
        mul_sem = stack.enter_context(nc.semaphore("mul_sem"))
        acc_sem = stack.enter_context(nc.semaphore("acc_sem"))
        out_sem = stack.enter_context(nc.semaphore("out_sem"))
        ys = stack.enter_context(nc.sbuf_tensor("ys", [P, W2], f16))
        acc = stack.enter_context(nc.sbuf_tensor("acc", [P, NTILES], f32))
        scr = stack.enter_context(nc.sbuf_tensor("scr", [P, 2], f32))

        def issue(eng, tiles):
            for t in tiles:
                lo, hi = OFFS[t], OFFS[t + 1]
                eng.dma_start(out=ys[:, lo:hi], in_=ym[:, lo:hi]).then_inc(tsem[t], 16)

        @block.sync
        def _(sync):
            issue(sync, RING_S)
            sync.wait_ge(acc_sem, NTILES - len(TAIL_VEC) + 1)
            # no wait on out_sem: the ~8us NEFF epilogue barrier runs after
            # the program ends, far longer than this 5KB DMA's ~2us landing
            sync.dma_start(out=out[:], in_=acc[:]).then_inc(out_sem, 16)

        @block.vector
        def _(vector):
            for i in range(NTILES):
                vector.wait_ge(tsem[i], 16)
                lo = OFFS[i]
                mid = lo + T_K[i]
                hi = OFFS[i + 1]
                nc.vector.tensor_mul(
                    out=ys[:, lo:mid], in0=ys[:, lo:mid], in1=ys[:, mid:hi]
                ).then_inc(mul_sem, 1)
                if i in VEC_ACC:
                    ts = nc.vector.tensor_scalar(
                        ys[:, lo:mid], ys[:, lo:mid], 1.0, 0.0,
                        op0=mybir.AluOpType.mult, op1=mybir.AluOpType.add,
                        accum_out=acc[:, i:i + 1],
                    )
                    if i not in TAIL_VEC:
                        ts.then_inc(acc_sem, 1)
            # flush guard: ordered after the tail TS ops' compiler-emitted
            # DVE_READ_ACCUMULATOR flushes; its inc releases the out-DMA
            nc.vector.tensor_copy(out=scr[:], in_=acc[:, 0:2]).then_inc(acc_sem, 1)

        @block.scalar
        def _(scalar):
            issue(scalar, RING_A)
            for i in range(NTILES):
                if i in VEC_ACC:
                    continue
                scalar.wait_ge(mul_sem, i + 1)
                lo = OFFS[i]
                mid = lo + T_K[i]
                hi = OFFS[i + 1]
                nc.scalar.activation(
                    out=ys[:, mid:hi],
                    in_=ys[:, lo:mid],
                    func=mybir.ActivationFunctionType.Copy,
                    accum_out=acc[:, i:i + 1],
                ).then_inc(acc_sem, 1)

    return nc


def _get_nc():
    global _CACHED_NC
    if _CACHED_NC is None:
        _CACHED_NC = _build_nc()
    return _CACHED_NC


def kernel(x, yOrig, mask):
    x = np.asarray(x)
    yOrig = np.asarray(yOrig, dtype=np.float32)
    mask = np.asarray(mask, dtype=np.float32)

    xs = float(x.reshape(-1)[0])
    ind = int(np.floor((xs - X0) / DX))
    shift = ind % N

    if shift == 0:
        rolled = mask
    else:
        rolled = np.concatenate([mask[N - shift:], mask[:N - shift]])

    yq = yOrig.astype(np.float16)
    mq = rolled.astype(np.float16)

    in_maps = []
    for c in range(NCORES):
        yr = yq[c * S:(c + 1) * S].reshape(P, F)
        mr = mq[c * S:(c + 1) * S].reshape(P, F)
        ymc = np.empty((P, W2), dtype=np.float16)
        for t in range(NTILES):
            a, b = STARTS[t], STARTS[t + 1]
            lo = OFFS[t]
            mid = lo + T_K[t]
            hi = OFFS[t + 1]
            ymc[:, lo:mid] = yr[:, a:b]
            ymc[:, mid:hi] = mr[:, a:b]
        in_maps.append({"ym": ymc})

    res = run_bass_kernel_spmd(_get_nc(), in_maps, core_ids=list(range(NCORES)))

    partials = np.concatenate([r["out"].reshape(-1) for r in res.results])
    total = np.float32(partials.sum(dtype=np.float32))

    if xs >= XMAX or xs < X0:
        total = np.float32(0.0)

    kernel.last_results = res
    return np.asarray(total, dtype=np.float32)
